# revision 71
# baseline (speedup 1.0000x reference)
"""Trainium2 Bass kernel for nn_BaconAdditionReasoner (histogram_binning).

Math (per batch row b):
    P1 = soft_perm(W1), P2 = soft_perm(W2)          (host, 10x10)
    l1 = p1 @ P1.T, l2 = p2 @ P2.T                  (host, tiny transform)
    u = log(1 - min(l1, 1-1e-6)), v = log(1 - min(l2, 1-1e-6))   (host)
    s[i,j] = min(l1[i], l2[j])
    log1m[i,j] = log(1 - s[i,j])  ==  max(u[i], v[j])
    logprod[k] = sum_{i+j=k} max(u_i, v_j)
              = sum_{i+j=k} u_i  +  sum_{i+j=k} relu(v_j - u_i)
    y = 1 - exp(logprod);  out = y / sum_k(y)       (host epilogue)

The device runs the histogram-binning core over 8 cores (32768 rows each),
features on partitions / batch on the free dim, 4 chunks of 512 batch rows on
32-aligned partition bands -> one supertile = 2048 rows.  All matmul operands
are fp16 (1 cyc/row on PE); validated ~2e-3 rel err vs the 2e-2 gate.
  - in-DMA:    one [116, 4*F] fp16 u/v transfer per 4 supertiles
  - D-matmul:  4 fp16 matmuls expand (v_j - u_i) pairs + pass -u through,
               one [110, F] PSUM quarter per chunk
  - relu:      one engine per quarter (VectorE q0/q2, ScalarE q1/q3)
  - A-matmul:  4 fp16 col-tiled matmuls reduce pairs over anti-diagonals
               (k=i+j) and add S_k, giving logprod; zero cols k=19..31
  - copy:      logprod PSUM -> SBUF fp16, column-split across both engines
  - out-DMA:   one [128, 4*F] fp16 transfer per 4 supertiles; host slices the
               19 real rows per 32-band, de-interleaves, then finishes with
               y = 1 - exp(logprod) and y / (y.sum(axis=1) + 1e-9).
Emission is software-pipelined (PE: D(t)x4, A(t-1)x4; ACT: copy_a(t-2),
relu q1/q3(t); DVE: copy_v(t-2), relu q0/q2(t)) so every engine's in-order
queue only reaches instructions whose inputs are already computed.
"""

import numpy as np

# ---------------------------------------------------------------- constants
B = 262144
NCORES = 8
BC = B // NCORES            # 32768 rows per core
F = 512                     # batch columns per chunk per supertile (HW max)
CH = 4                      # chunks per supertile (32-aligned partition bands)
ROWS_ST = F * CH            # 2048 rows per supertile
NST = BC // ROWS_ST         # 16 supertiles per core
NCOLS = NST * F             # 8192 columns in pc / yraw
BATCH = 4                   # supertiles per DMA batch

PCROWS = 116                # 3*32 + 20 partitions carrying the banded input
KD = 110                    # pair rows (100) + passthrough -u rows (10)

# fp16 const (lhsT) column layout inside wk16
WD0, WD1 = 0, 110           # D weights  [20, 110], replicated per 32-band
WA0, WA1 = 110, 142         # A weights  [110, 32]
W16C = 142

# copy column split: VectorE gets [0:CC), ScalarE [CC:F)  (DVE's relu
# quarters are the early-ready ones, so it gets less head copy work)
CC = 208


def _soft_perm_np(W: np.ndarray) -> np.ndarray:
    W = W.astype(np.float32)
    lo = W.min(axis=1, keepdims=True)
    hi = W.max(axis=1, keepdims=True)
    Wn = (W - lo) / (hi - lo + np.float32(1e-8))
    return Wn / (Wn.sum(axis=1, keepdims=True) + np.float32(1e-8))


def _build_wk16() -> np.ndarray:
    wk = np.zeros((128, W16C), dtype=np.float32)
    # --- D: pairs m=10i+j get v_j - u_i ; cols 100..109 pass -u through
    d = np.zeros((20, KD), dtype=np.float32)
    for i in range(10):
        for j in range(10):
            d[i, 10 * i + j] = -1.0
            d[10 + j, 10 * i + j] = 1.0
    for e in range(10):
        d[e, 100 + e] = -1.0
    for q in range(4):
        wk[32 * q : 32 * q + 20, WD0:WD1] = d
    # --- A: [110, 32]; rows m<100: +1 at k=i+j ; rows 100+e: -1 for
    #     k in [e, e+9] (those rows hold -u, so -1 gives +u)
    a = np.zeros((KD, 32), dtype=np.float32)
    for i in range(10):
        for j in range(10):
            a[10 * i + j, i + j] = 1.0
    for e in range(10):
        a[100 + e, e : e + 10] = -1.0
    wk[0:KD, WA0:WA1] = a
    return wk.astype(np.float16)


def _build_pc(uc: np.ndarray, vc: np.ndarray) -> np.ndarray:
    """u/v [BC,10] -> pc [PCROWS, NCOLS] fp16: row 32q+e = u_e (e<10) or
    v_{e-10} of chunk-band q; col F*s+f = batch row ROWS_ST*s + F*q + f."""
    uu = uc.reshape(NST, CH, F, 10).transpose(1, 3, 0, 2).reshape(CH, 10, NCOLS)
    vv = vc.reshape(NST, CH, F, 10).transpose(1, 3, 0, 2).reshape(CH, 10, NCOLS)
    pc = np.zeros((PCROWS, NCOLS), dtype=np.float16)
    for q in range(CH):
        pc[32 * q : 32 * q + 10] = uu[q]
        pc[32 * q + 10 : 32 * q + 20] = vv[q]
    return pc


def _unpack_yraw(yraw: np.ndarray) -> np.ndarray:
    """yraw [128, NCOLS] fp16 (logprod) -> y [BC, 19] f32."""
    t = yraw.reshape(4, 32, NST, F).transpose(2, 0, 3, 1)  # [s, g, f, 32]
    lp = np.ascontiguousarray(t.reshape(BC, 32)[:, :19]).astype(np.float32)
    y = 1.0 - np.exp(lp)
    return y / (y.sum(axis=1, keepdims=True) + np.float32(1e-9))


def _patch_act_tables():
    """Force Ln/Exp/Abs to resolve to the single set that has all three
    (natural_log_exp_and_others) so the greedy per-function chooser cannot
    ping-pong table loads mid-kernel."""
    import concourse.bacc as bacc
    from concourse import mybir

    if getattr(bacc, "_act_tables_patched", False):
        return
    orig = bacc.get_activation_tables
    AF = mybir.ActivationFunctionType
    shared = {AF.Ln, AF.Exp, AF.Abs}

    def patched(arch):
        tabs = orig(arch)
        if "natural_log_exp_and_others" in tabs:
            for name, funcs in tabs.items():
                if name != "natural_log_exp_and_others":
                    tabs[name] = set(funcs) - shared
        return tabs

    bacc.get_activation_tables = patched
    bacc._act_tables_patched = True


def build_bass():
    import concourse.bass as bass
    import concourse.bacc as bacc
    import concourse.tile as tile
    from concourse import mybir

    _patch_act_tables()
    f32 = mybir.dt.float32
    f16 = mybir.dt.float16
    AF = mybir.ActivationFunctionType
    ALU = mybir.AluOpType

    nc = bacc.Bacc("TRN2", target_bir_lowering=False)

    pc_d = nc.dram_tensor("pc", [PCROWS, NCOLS], f16, kind="ExternalInput")
    wk_d = nc.dram_tensor("wk", [128, W16C], f16, kind="ExternalInput")
    y_d = nc.dram_tensor("yraw", [128, NCOLS], f16, kind="ExternalOutput")

    with tile.TileContext(nc) as tc:
        with (
            tc.tile_pool(name="singles", bufs=1) as singles,
            tc.tile_pool(name="pack", bufs=2) as pack_p,
            tc.tile_pool(name="kt", bufs=2) as kt_p,
            tc.tile_pool(name="oo", bufs=2) as oo_p,
            tc.tile_pool(name="psD", bufs=6, space="PSUM") as psD,
            tc.tile_pool(name="psA", bufs=2, space="PSUM") as psA,
        ):
            wk = singles.tile([128, W16C], f16)

            packs = {}
            oos = {}
            kts = {}

            def dma_in(b):
                pack = packs[b] = pack_p.tile(
                    [PCROWS, BATCH * F], f16, name="pack"
                )
                nc.sync.dma_start(
                    pack[:, :],
                    pc_d[:, b * BATCH * F : (b + 1) * BATCH * F],
                )

            def emit_D(s):
                pack = packs[s // BATCH]
                c0 = (s % BATCH) * F
                dps = []
                for q in range(4):
                    dp = psD.tile([KD, F], f32, name="dp")
                    nc.tensor.matmul(
                        dp[0:KD, :],
                        wk[32 * q : 32 * q + 20, WD0:WD1],
                        pack[32 * q : 32 * q + 20, c0 : c0 + F],
                        start=True, stop=True,
                        tile_position=(32 * q, 0),
                    )
                    dps.append(dp)
                return dps

            def emit_relu(s, dps):
                kt = kts[s] = kt_p.tile([KD, CH * F], f16, name="kt")
                # one engine per quarter: DVE takes the early-ready q0/q2,
                # ACT q1/q3 (emission order = queue order per engine)
                for q, eng in ((0, "v"), (1, "a"), (2, "v"), (3, "a")):
                    dst = kt[:, q * F : (q + 1) * F]
                    if eng == "a":
                        nc.scalar.activation(dst, dps[q][:, :], AF.Relu)
                    else:
                        nc.vector.tensor_scalar(
                            dst, dps[q][:, :], 0.0, None, op0=ALU.max
                        )

            def emit_A(s):
                kt = kts.pop(s)
                ap_ = psA.tile([128, F], f32)
                for g in range(4):
                    nc.tensor.matmul(
                        ap_[32 * g : 32 * g + 32, :],
                        wk[0:KD, WA0:WA1],
                        kt[0:KD, g * F : (g + 1) * F],
                        start=True, stop=True,
                        tile_position=(0, 32 * g),
                    )
                return ap_

            def emit_copy(s, ap_):
                bi = s % BATCH
                if bi == 0:
                    oos[s // BATCH] = oo_p.tile(
                        [128, BATCH * F], f16, name="oo"
                    )
                dst = oos[s // BATCH][:, bi * F : (bi + 1) * F]
                if s == NST - 1:
                    # tail: single engine so the final DMA isn't gated on
                    # the slower-scheduled ACT half
                    nc.vector.tensor_copy(dst, ap_[:, :])
                else:
                    nc.vector.tensor_copy(dst[:, 0:CC], ap_[:, 0:CC])
                    nc.scalar.activation(dst[:, CC:F], ap_[:, CC:F], AF.Copy)

            def dma_out(b):
                oo = oos.pop(b)
                if b == NST // BATCH - 1:
                    # split so the tail drains as each supertile completes
                    for j in range(BATCH):
                        nc.sync.dma_start(
                            y_d[:, (b * BATCH + j) * F : (b * BATCH + j + 1) * F],
                            oo[:, j * F : (j + 1) * F],
                        )
                else:
                    nc.sync.dma_start(
                        y_d[:, b * BATCH * F : (b + 1) * BATCH * F], oo[:, :]
                    )

            # prefetch: first F input columns on the SP queue while the
            # weights go out on the Pool queue (fixed DMA overheads overlap;
            # only the short transfers serialize on the DMA engines)
            pack0 = packs[0] = pack_p.tile([PCROWS, BATCH * F], f16, name="pack0")
            nc.sync.dma_start(pack0[:, 0:F], pc_d[:, 0:F])
            nc.gpsimd.dma_start(wk[:, :], wk_d[:, :])
            nc.sync.dma_start(pack0[:, F : 2 * F], pc_d[:, F : 2 * F])
            nc.sync.dma_start(pack0[:, 2 * F : 3 * F], pc_d[:, 2 * F : 3 * F])
            nc.sync.dma_start(
                pack0[:, 3 * F : BATCH * F], pc_d[:, 3 * F : BATCH * F]
            )

            # warm the PE p-state ramp while the first DMAs land: junk
            # matmuls on a zeroed tile, outputs never read
            junk = singles.tile([128, F], f16)
            nc.vector.memset(junk[:, :], 0.0)
            for _ in range(4):
                wp = psD.tile([KD, F], f32, name="dp")
                nc.tensor.matmul(
                    wp[:, :], junk[:, 0:KD], junk[:, :],
                    start=True, stop=True,
                )


            aps = {}
            for t in range(-1, NST + 1):
                s4, s0, sA, sC = t + 4, t, t - 1, t - 2
                if 0 <= s4 < NST and s4 % BATCH == 0 and s4 > 0:
                    dma_in(s4 // BATCH)
                if 0 <= sC < NST:
                    emit_copy(sC, aps.pop(sC))
                if 0 <= s0 < NST:
                    emit_relu(s0, emit_D(s0))
                if 0 <= sA < NST:
                    aps[sA] = emit_A(sA)
                if 0 <= sC < NST and sC % BATCH == BATCH - 1:
                    dma_out(sC // BATCH)
                if sA == NST - 1:
                    # epilogue: the last copy + DMA need not wait extra ticks
                    emit_copy(NST - 1, aps.pop(NST - 1))
                    dma_out((NST - 1) // BATCH)
    nc.compile()
    return nc


_NC_CACHE = None


def kernel(p1, p2, W1, W2):
    global _NC_CACHE
    from concourse.bass_utils import run_bass_kernel_spmd

    P1n = _soft_perm_np(np.asarray(W1))
    P2n = _soft_perm_np(np.asarray(W2))
    wk16 = _build_wk16()
    p1 = np.asarray(p1, dtype=np.float32)
    p2 = np.asarray(p2, dtype=np.float32)
    # tiny 10x10 input transform + log, mirroring the reference's hi clamp
    l1 = np.minimum(p1 @ P1n.T, np.float32(1.0 - 1e-6))
    l2 = np.minimum(p2 @ P2n.T, np.float32(1.0 - 1e-6))
    u = np.log1p(-l1)
    v = np.log1p(-l2)

    in_maps = []
    for c in range(NCORES):
        sl = slice(c * BC, (c + 1) * BC)
        in_maps.append({"pc": _build_pc(u[sl], v[sl]), "wk": wk16})

    if _NC_CACHE is None:
        _NC_CACHE = build_bass()
    res = run_bass_kernel_spmd(_NC_CACHE, in_maps, core_ids=list(range(NCORES)))
    out = np.concatenate(
        [_unpack_yraw(res.results[c]["yraw"]) for c in range(NCORES)], axis=0
    )
    return out
